# revision 23
# baseline (speedup 1.0000x reference)
"""GroupedQueryAttention on 8 Trainium2 NeuronCores (v5).

Sharding: core c = 4*b + r handles batch b (of 2) and token chunk r (512
of 2048 tokens) for Q/attention/o_proj over ALL 16 heads. K/V projections
are sharded by KV group: core r computes group g=r's K/V for all T, then
chunk-split AllGathers across each batch's 4 cores make every core
independent for the rest of the kernel -- no output collective.

v3 deltas over the 492us baseline:
  - phase 1 runs chunks in payload order [own,1,2,3] and pushes each
    chunk's K/V payload immediately; the AllGather is split in
    AG_SPLIT pieces issued as soon as their chunks are done, so the
    collective overlaps phase 1 + Q-proj instead of trailing them.
  - the 8 MB wq stream lives on the scalar HWDGE queue, 11 tiles deep
    (tiles 0-10 prefetch with no buffer-reuse waits), so Q-proj is
    matmul-paced instead of DMA-handoff-paced: the baseline lost ~50us
    to wq starvation and a 6-deep stream still cost ~1.5us/head.
  - the 64 attention-output transposes run as XBAR dma_start_transpose
    ([128,512] -> [128,4,128] blocked transpose) on the sync queue, off
    the PE; phase-1 V blocks keep PE transposes (their deps resolve too
    late for an in-order DMA queue and would head-of-line block it).
    start=True clears a whole PSUM bank's has_written bits, so only the
    first accumulation group in a shared bank asserts it.
    (fp8 DoubleRow was tried and fails the 2e-2 gate: quantizing any of
    x/Wq/Wk/Wv/A/Wo to e4m3 alone costs 3.6e-2..5.5e-2 max-rel error;
    an interleaved Q-proj/attention schedule reached 388us second-call
    but regressed first-call latency to 470-560us on cold collectives,
    so this stable schedule is kept.)
  - o_proj is nb-outer: Wo streams per-nb [128, h, 512] work tiles on
    the gpsimd queue (the first two token-gated on late Q-proj outputs;
    a dep-free 2 MB DMA gets scheduled at t=0 and starves the init/x/wq
    streams for HBM), bias is a DVE add fused into the PSUM drain, and
    out is written fp16 (host upcasts). Scalar queue carries no DMA
    during attention so exp (the ~140us ACT-bound softmax, 95% busy in
    its window) never waits.
  - PSUM: tag "big" = 3 x [128,1024] (scores 3-deep, o_proj nb-pairs),
    tag "half" = 2 x [128,512]-sized slots holding K/V/Q accumulators
    and the paired A@V accumulators [128,258] (2 x (128 out + 1 denom)).

All matmuls fp16 (1 PE cycle/row) with fp32 PSUM accumulation. Layouts
avoid transposing the big P matrix: projections produce Q^T/K^T/V^T
directly; scores are S^T = (K^T block).T @ Q^T; exp(S^T) = P^T feeds
A@V as the stationary; V carries a ones-column so the softmax
denominator falls out of the A@V matmul for free.
"""

import math
import sys

import numpy as np

sys.path.insert(0, "/opt/trn_rl_repo")

B = 2
T = 2048
D = 2048
HEADS = 16
GROUPS = 4
HD = 128  # head dim
M = HEADS // GROUPS  # heads per group = 4
SCALE = 1.0 / math.sqrt(HD)
N_CORES = 8
TCH = 512  # token chunk per core
NTCH = T // TCH  # 4
NSB = T // 128  # 16 key blocks
NKS = D // 128  # 16 contraction steps for projections
NQ = NKS // 4  # 4 quad blocks for the x stream
NNB = D // TCH  # 4 o_proj output column blocks

NWQB = 11  # wq stream depth (tiles 0-10 prefetch without pacing stalls)
AG_SPLIT = 2  # number of chunk-split AllGathers
CPA = NTCH // AG_SPLIT  # chunks per AllGather

_COMPILED = {}


def _build():
    import concourse.bass as bass
    import concourse.mybir as mybir
    import concourse.tile as tile
    from concourse import bacc
    from concourse.masks import make_identity

    f16 = mybir.dt.float16
    f32 = mybir.dt.float32
    Exp = mybir.ActivationFunctionType.Exp
    Mult = mybir.AluOpType.mult
    Add = mybir.AluOpType.add

    nc = bacc.Bacc("TRN2", target_bir_lowering=False, num_devices=N_CORES)

    # x^T as (slot, quad) row-blocks of [128, 2048]; slot 0 = own chunk
    xcb_d = nc.declare_dram_parameter("xcb", [NTCH * NQ * 128, 4 * TCH], f16,
                                      isOutput=False)
    wq_d = nc.declare_dram_parameter("wq", [128, HEADS * NKS * 128], f16,
                                     isOutput=False)
    wk_d = nc.declare_dram_parameter("wk", [128, NKS * 128], f16, isOutput=False)
    wv_d = nc.declare_dram_parameter("wv", [128, NKS * 128], f16, isOutput=False)
    # nb-major layout: [128, nb, h, 512]
    wo_d = nc.declare_dram_parameter("wo", [128, HEADS * NNB * TCH], f16,
                                     isOutput=False)
    bqs_d = nc.declare_dram_parameter("bqs", [128, HEADS], f32, isOutput=False)
    bks_d = nc.declare_dram_parameter("bks", [128, 1], f32, isOutput=False)
    bvs_d = nc.declare_dram_parameter("bvs", [128, 1], f32, isOutput=False)
    bob_d = nc.declare_dram_parameter("bob", [128, D], f16, isOutput=False)
    out_d = nc.declare_dram_parameter("out", [TCH, D], f16, isOutput=True)

    groups = [[0, 1, 2, 3], [4, 5, 6, 7]]

    with tile.TileContext(nc) as tc:
        with (
            tc.tile_pool(name="const", bufs=1) as const,
            tc.tile_pool(name="work", bufs=2) as work,
            tc.tile_pool(name="psum", bufs=1, space="PSUM") as psum,
            tc.tile_pool(name="dram", bufs=1, space="DRAM") as dram,
        ):
            ident = const.tile([128, 128], f16)
            make_identity(nc, ident)
            bqs = const.tile([128, HEADS], f32)
            bks = const.tile([128, 1], f32)
            bvs = const.tile([128, 1], f32)
            bob = const.tile([128, D], f16)

            wk_sb = const.tile([128, NKS, 128], f16)
            wv_sb = const.tile([128, NKS, 128], f16)
            x_own = const.tile([128, NQ, 4 * TCH], f16)

            kt = const.tile([128, GROUPS, T], f16)  # gathered K^T
            v_sb = const.tile([128, GROUPS, NSB, 132], f16)  # gathered V + ones
            qt = const.tile([128, HEADS, TCH], f16)  # own-chunk Q^T
            at = const.tile([128, HEADS, 4, 128], f16)  # own-chunk A^T
            nc.vector.memset(v_sb[:, :, :, 128:129], 1.0)

            # initial loads, quad-interleaved so chunk 0 can start early
            for q in range(NQ):
                nc.scalar.dma_start(wk_sb[:, q * 4 : (q + 1) * 4, :],
                                    wk_d[:, q * 512 : (q + 1) * 512])
                nc.scalar.dma_start(wv_sb[:, q * 4 : (q + 1) * 4, :],
                                    wv_d[:, q * 512 : (q + 1) * 512])
                nc.sync.dma_start(x_own[:, q, :],
                                  xcb_d[q * 128 : (q + 1) * 128, :])
            nc.scalar.dma_start(bks[:], bks_d[:])
            nc.scalar.dma_start(bvs[:], bvs_d[:])
            nc.scalar.dma_start(bqs[:], bqs_d[:])

            # wq stream on the scalar HWDGE queue
            wq_tiles = {}

            def issue_wq(h):
                if h >= HEADS or h in wq_tiles:
                    return
                wqh = work.tile([128, NKS * 128], f16, tag="wq", bufs=NWQB,
                                name="wqh", uniquify=True)
                nc.scalar.dma_start(
                    wqh[:], wq_d[:, h * NKS * 128 : (h + 1) * NKS * 128])
                wq_tiles[h] = wqh

            issue_wq(0)
            issue_wq(1)

            # ---- phase 1: K/V projection for own group, all T ----
            kvl = {}
            kvg = {}
            for a in range(AG_SPLIT):
                kvl[a] = dram.tile([256, CPA * TCH], f16, tag=f"kvl{a}",
                                   name=f"kvl{a}")
                kvg[a] = dram.tile([4 * 256, CPA * TCH], f16, tag=f"kvg{a}",
                                   name=f"kvg{a}")

            for c in range(NTCH):
                kacc = psum.tile([128, TCH], f32, tag="half", bufs=2, name="kacc")
                vacc = psum.tile([128, TCH], f32, tag="half", bufs=2, name="vacc")
                for q in range(NQ):
                    if c == 0:
                        x4 = x_own[:, q, :]
                    else:
                        x4t = work.tile([128, 4 * TCH], f16, tag="xs", bufs=4,
                                        name="x4t")
                        nc.sync.dma_start(
                            x4t[:],
                            xcb_d[(c * NQ + q) * 128 : (c * NQ + q + 1) * 128, :],
                        )
                        x4 = x4t[:]
                    for k2 in range(4):
                        ks = q * 4 + k2
                        xb = x4[:, k2 * TCH : (k2 + 1) * TCH]
                        nc.tensor.matmul(
                            kacc[:], wk_sb[:, ks, :], xb,
                            start=(ks == 0), stop=(ks == NKS - 1),
                        )
                        nc.tensor.matmul(
                            vacc[:], wv_sb[:, ks, :], xb,
                            start=(ks == 0), stop=(ks == NKS - 1),
                        )
                a, cc = c // CPA, c % CPA
                ktc = work.tile([128, TCH], f16, tag="ktc", bufs=2, name="ktc")
                nc.vector.tensor_scalar_add(ktc[:], kacc[:], bks[:, 0:1])
                nc.scalar.dma_start(kvl[a][0:128, cc * TCH : (cc + 1) * TCH],
                                    ktc[:])
                vtc = work.tile([128, TCH], f16, tag="vtc", bufs=2, name="vtc")
                nc.vector.tensor_scalar_add(vtc[:], vacc[:], bvs[:, 0:1])
                # PE transpose per 128-block (an XBAR dma-transpose here
                # head-of-line blocks the scalar queue on chunk deps)
                vn = work.tile([128, 4, 128], f16, tag="vt", bufs=2, name="vn")
                for sb in range(4):
                    tp = psum.tile([128, 128], f16, tag="big", bufs=3, name="tp")
                    nc.tensor.transpose(tp[:], vtc[:, sb * 128 : (sb + 1) * 128],
                                        ident[:])
                    nc.vector.tensor_copy(vn[:, sb, :], tp[:])
                nc.scalar.dma_start(kvl[a][128:256, cc * TCH : (cc + 1) * TCH],
                                    vn[:])
                # cap at tile NWQB-1: deeper tiles would wait on Q-proj
                # consumption and head-of-line block the scalar queue
                issue_wq(min(2 + 2 * c, NWQB - 1))
                issue_wq(min(3 + 2 * c, NWQB - 1))
                if cc == CPA - 1:
                    nc.gpsimd.collective_compute(
                        "AllGather", mybir.AluOpType.bypass,
                        replica_groups=groups,
                        ins=[kvl[a][:]], outs=[kvg[a][:]],
                    )

            issue_wq(NWQB - 1)  # last unpaced tile

            # ---- phase 2: Q projection (own chunk, all heads); overlaps AG ----
            for h in range(HEADS):
                wqh = wq_tiles[h]
                # "big" slots are idle during Q-proj: 3-deep head pipelining
                qacc = psum.tile([128, TCH], f32, tag="big", bufs=3, name="qacc")
                for ks in range(NKS):
                    nc.tensor.matmul(
                        qacc[:], wqh[:, ks * 128 : (ks + 1) * 128],
                        x_own[:, ks // 4, (ks % 4) * TCH : (ks % 4 + 1) * TCH],
                        start=(ks == 0), stop=(ks == NKS - 1),
                    )
                nc.vector.tensor_scalar(
                    qt[:, h, :], qacc[:], SCALE, bqs[:, h : h + 1],
                    op0=Mult, op1=Add,
                )
                issue_wq(h + NWQB)

            nc.scalar.dma_start(bob[:], bob_d[:])

            # unpack gathered K^T / V into SBUF; groups 0/1 on sync run as
            # soon as each AG lands, groups 2/3 ride scalar behind the wq
            # stream (resolved well before their attention turn)
            for g in range(GROUPS):
                eng = nc.sync if g < 2 else nc.scalar
                for a in range(AG_SPLIT):
                    eng.dma_start(
                        kt[:, g, a * CPA * TCH : (a + 1) * CPA * TCH],
                        kvg[a][g * 256 : g * 256 + 128, :],
                    )
                    eng.dma_start(
                        v_sb[:, g, a * CPA * 4 : (a + 1) * CPA * 4, 0:128],
                        kvg[a][g * 256 + 128 : (g + 1) * 256, :],
                    )

            # ---- phase 3: attention for own chunk, all heads ----
            for g in range(GROUPS):
                for hh in range(M):
                    h = g * M + hh
                    opk01 = psum.tile([128, 258], f32, tag="half", bufs=2,
                                      name="opk01")
                    opk23 = psum.tile([128, 258], f32, tag="half", bufs=2,
                                      name="opk23")
                    opks = [(opk01, 0), (opk01, 129), (opk23, 0), (opk23, 129)]
                    for sp in range(NSB // 2):
                        sps2 = psum.tile([128, 2 * TCH], f32, tag="big", bufs=3,
                                         name="sps2")
                        for j in range(2):
                            s = sp * 2 + j
                            nc.tensor.matmul(
                                sps2[:, j * TCH : (j + 1) * TCH],
                                kt[:, g, s * 128 : (s + 1) * 128], qt[:, h, :],
                                start=True, stop=True,
                            )
                        p2 = work.tile([128, 2 * TCH], f16, tag="p", bufs=4,
                                       name="p2")
                        nc.scalar.activation(p2[:], sps2[:], Exp)
                        for j in range(2):
                            s = sp * 2 + j
                            for tb in range(4):
                                opk, off = opks[tb]
                                # start=True clears the WHOLE PSUM bank's
                                # has_written bits, so only the first group
                                # in each shared bank may assert it; the
                                # off=129 group's s=0 matmul writes fresh
                                # (per-element has_written=0) with start=False
                                nc.tensor.matmul(
                                    opk[:, off : off + 129],
                                    p2[:, j * TCH + tb * 128
                                       : j * TCH + (tb + 1) * 128],
                                    v_sb[:, g, s, 0:129],
                                    start=(s == 0 and off == 0),
                                    stop=(s == NSB - 1),
                                    skip_group_check=(off != 0),
                                )
                    o_sb = work.tile([128, TCH], f16, tag="osb", bufs=2,
                                     name="o_sb")
                    for tb in range(4):
                        opk, off = opks[tb]
                        rcp = work.tile([128, 1], f32, tag="rcp", bufs=4,
                                        name="rcp")
                        nc.vector.reciprocal(rcp[:], opk[:, off + 128 : off + 129])
                        nc.vector.tensor_scalar_mul(
                            o_sb[:, tb * 128 : (tb + 1) * 128],
                            opk[:, off : off + 128], rcp[:])
                    # at[:, h, tb, :] = o_sb[:, tb*128:+128].T via XBAR
                    nc.sync.dma_start_transpose(at[:, h], o_sb[:])

            # ---- phase 4: o_proj for own chunk, full D ----
            # nb-outer: Wo streamed per-nb [128, h, 512] on gpsimd (queued
            # behind the AG issues, so it cannot start before ~phase-1 end)
            for nb in range(NNB):
                wob = work.tile([128, HEADS, TCH], f16, tag="wo", bufs=2,
                                name="wob")
                if nb < 2:
                    # token gate: without a dep the scheduler runs this 2 MB
                    # DMA at t=0, starving the init/x/wq streams for HBM
                    nc.vector.tensor_copy(wob[:, 0, 0:1],
                                          qt[:, 12 + nb, 0:1])
                nc.gpsimd.dma_start(
                    wob[:], wo_d[:, nb * HEADS * TCH : (nb + 1) * HEADS * TCH])
                pp01 = psum.tile([128, 2 * TCH], f32, tag="big", bufs=3,
                                 name="pp01")
                pp23 = psum.tile([128, 2 * TCH], f32, tag="big", bufs=3,
                                 name="pp23")
                pps = [(pp01, 0), (pp01, TCH), (pp23, 0), (pp23, TCH)]
                for h in range(HEADS):
                    for tb in range(4):
                        pp, off = pps[tb]
                        nc.tensor.matmul(
                            pp[:, off : off + TCH],
                            at[:, h, tb, :],
                            wob[:, h, :],
                            start=(h == 0), stop=(h == HEADS - 1),
                        )
                for tb in range(4):
                    pp, off = pps[tb]
                    ob = work.tile([128, TCH], f16, tag="ob", bufs=4, name="ob")
                    nc.vector.scalar_tensor_tensor(
                        ob[:], pp[:, off : off + TCH], 1.0,
                        bob[:, nb * TCH : (nb + 1) * TCH],
                        op0=Mult, op1=Add,
                    )
                    nc.sync.dma_start(
                        out_d[tb * 128 : (tb + 1) * 128,
                              nb * TCH : (nb + 1) * TCH],
                        ob[:],
                    )

    nc.compile()
    return nc


def _get_nc():
    if "nc" not in _COMPILED:
        _COMPILED["nc"] = _build()
    return _COMPILED["nc"]


def kernel(x, Wq, bq, Wk, bk, Wv, bv, Wo, bo):
    from concourse.bass_utils import run_bass_kernel_spmd

    x = np.asarray(x, np.float32)
    Wq = np.asarray(Wq, np.float32)
    Wk = np.asarray(Wk, np.float32)
    Wv = np.asarray(Wv, np.float32)
    Wo = np.asarray(Wo, np.float32)
    bq = np.asarray(bq, np.float32)
    bk = np.asarray(bk, np.float32)
    bv = np.asarray(bv, np.float32)
    bo = np.asarray(bo, np.float32)

    nc = _get_nc()

    # shared across cores
    wq_h = np.ascontiguousarray(
        Wq.reshape(NKS, 128, HEADS, 128).transpose(1, 2, 0, 3).reshape(128, -1)
    ).astype(np.float16)
    # nb-major o_proj layout [128, nb, h, 512]
    wo_h = np.ascontiguousarray(
        Wo.reshape(HEADS, 128, NNB, TCH).transpose(1, 2, 0, 3).reshape(128, -1)
    ).astype(np.float16)
    bqs_h = np.ascontiguousarray((bq * SCALE).reshape(HEADS, 128).T)
    bob_h = np.ascontiguousarray(np.broadcast_to(bo.astype(np.float16), (128, D)))
    # x^T per batch, pre-blocked into (chunk, quad) [128, 2048] row-blocks
    xq16 = []
    for b in range(B):
        xTb = x[b].T.astype(np.float16)  # [D, T]
        blocks = xTb.reshape(NKS, 128, NTCH, TCH).transpose(2, 0, 1, 3)
        # [chunk, ks, 128, TCH] -> quads: [chunk, quad, 128, 4*TCH]
        blocks = blocks.reshape(NTCH, NQ, 4, 128, TCH).transpose(0, 1, 3, 2, 4)
        xq16.append(np.ascontiguousarray(blocks.reshape(NTCH, NQ * 128, 4 * TCH)))

    wk_g, wv_g, bks_g, bvs_g = [], [], [], []
    for g in range(GROUPS):
        wk_g.append(
            np.ascontiguousarray(
                Wk[:, g * HD : (g + 1) * HD].reshape(NKS, 128, HD)
                .transpose(1, 0, 2).reshape(128, -1)
            ).astype(np.float16)
        )
        wv_g.append(
            np.ascontiguousarray(
                Wv[:, g * HD : (g + 1) * HD].reshape(NKS, 128, HD)
                .transpose(1, 0, 2).reshape(128, -1)
            ).astype(np.float16)
        )
        bks_g.append(np.ascontiguousarray(bk[g * HD : (g + 1) * HD].reshape(1, HD).T))
        bvs_g.append(np.ascontiguousarray(bv[g * HD : (g + 1) * HD].reshape(1, HD).T))

    in_maps = []
    for c in range(N_CORES):
        b, r = c // 4, c % 4
        order = [r] + [i for i in range(NTCH) if i != r]
        xcb = np.concatenate([xq16[b][s] for s in order], axis=0)
        in_maps.append(
            {
                "xcb": np.ascontiguousarray(xcb),
                "wq": wq_h,
                "wk": wk_g[r],
                "wv": wv_g[r],
                "wo": wo_h,
                "bqs": bqs_h,
                "bks": bks_g[r],
                "bvs": bvs_g[r],
                "bob": bob_h,
            }
        )

    res = run_bass_kernel_spmd(nc, in_maps, list(range(N_CORES)))
    _COMPILED["last_res"] = res

    out = np.empty((B, T, D), np.float32)
    for b in range(B):
        for r in range(NTCH):
            out[b, r * TCH : (r + 1) * TCH, :] = (
                res.results[4 * b + r]["out"].astype(np.float32)
            )
    return out
